# revision 6
# baseline (speedup 1.0000x reference)
"""MLA prefill attention kernel for 8 Trainium2 NeuronCores.

Sharding: tensor-parallel over heads (4 of 32 heads per core), wo sharded on
its input dim; per-core partial outputs are summed on the host (the unshard
step of the row-parallel output projection).

Problem shapes (hardcoded per contract):
  B=1, T=2048, DIM=2048, H=32, D_NOPE=D_ROPE=16, D_QK=32, D_V=128, LATENT=128
"""

import math

import ml_dtypes
import numpy as np

T = 2048
DIM = 2048
H = 32
N_CORES = 8
HL = H // N_CORES  # 4 local heads
D_NOPE, D_ROPE = 16, 16
D_QK = D_NOPE + D_ROPE  # 32
D_V = 128
LAT = 128
ROPE_FACTOR, MSCALE0 = 40.0, 1.0
_ms = 0.1 * MSCALE0 * math.log(ROPE_FACTOR) + 1.0
SCALE = (D_QK**-0.5) * _ms * _ms

BF16 = ml_dtypes.bfloat16

# stream_shuffle source map (within each 32-partition block): rows 0..15
# identity (nope), rows 16..31 swap adjacent pairs (rope x0<->x1).
SWAP_MASK = [i if i < 16 else i ^ 1 for i in range(32)]

_CACHE: dict = {}


def _build_module():
    import concourse.mybir as mybir
    import concourse.tile as tile
    from concourse import bacc

    fp32 = mybir.dt.float32
    bf16 = mybir.dt.bfloat16
    Exp = mybir.ActivationFunctionType.Exp

    nc = bacc.Bacc("TRN2", target_bir_lowering=False, debug=False,
                   num_devices=N_CORES)

    # DRAM I/O (per-core shards prepared on host)
    xt_d = nc.dram_tensor("xt", (DIM, T), bf16, kind="ExternalInput").ap()
    wq_d = nc.dram_tensor("wq", (128, DIM), bf16, kind="ExternalInput").ap()
    wu_d = nc.dram_tensor("wu", (128, DIM), bf16, kind="ExternalInput").ap()
    wkvk_d = nc.dram_tensor("wkvk", (LAT, HL * D_QK), bf16, kind="ExternalInput").ap()
    wkvv_d = nc.dram_tensor("wkvv", (LAT, HL * D_V), bf16, kind="ExternalInput").ap()
    wo_d = nc.dram_tensor("wo", (128, 4 * T), bf16, kind="ExternalInput").ap()
    cq_d = nc.dram_tensor("cq", (128, T), bf16, kind="ExternalInput").ap()
    sq_d = nc.dram_tensor("sq", (128, T), bf16, kind="ExternalInput").ap()
    ck_d = nc.dram_tensor("ck", (128, T), bf16, kind="ExternalInput").ap()
    sk_d = nc.dram_tensor("sk", (128, T), bf16, kind="ExternalInput").ap()
    onesc_d = nc.dram_tensor("onesc", (128, 1), bf16, kind="ExternalInput").ap()
    onesr_d = nc.dram_tensor("onesr", (1, 128), fp32, kind="ExternalInput").ap()
    out_d = nc.dram_tensor("out", (T, T), fp32, kind="ExternalOutput").ap()

    with tile.TileContext(nc) as tc:
        with (
            tc.tile_pool(name="xin", bufs=1) as xin,
            tc.tile_pool(name="wts", bufs=1) as wts,
            tc.tile_pool(name="big", bufs=1) as big,
            tc.tile_pool(name="rope", bufs=1) as ropep,
            tc.tile_pool(name="pp", bufs=3) as ppool,
            tc.tile_pool(name="norm", bufs=2) as npool,
            tc.tile_pool(name="psA", bufs=2, space="PSUM") as psA,
            tc.tile_pool(name="psB", bufs=2, space="PSUM") as psB,
            tc.tile_pool(name="psC", bufs=2, space="PSUM") as psC,
        ):
            # ---- load inputs ----
            xc = []
            for kc in range(16):
                t = xin.tile([128, T], bf16, tag=f"xc{kc}")
                nc.sync.dma_start(t[:], xt_d[kc * 128:(kc + 1) * 128, :])
                xc.append(t)

            def load(pool, dram, shape, dt, tag):
                t = pool.tile(list(shape), dt, tag=tag, name=tag)
                nc.sync.dma_start(t[:], dram[:])
                return t

            wq_sb = load(wts, wq_d, (128, DIM), bf16, "wq")
            wu_sb = load(wts, wu_d, (128, DIM), bf16, "wu")
            wkvk_sb = load(wts, wkvk_d, (LAT, HL * D_QK), bf16, "wkvk")
            wkvv_sb = load(wts, wkvv_d, (LAT, HL * D_V), bf16, "wkvv")
            wo_sb = load(wts, wo_d, (128, 4 * T), bf16, "wo")
            cq_sb = load(wts, cq_d, (128, T), bf16, "cq")
            sq_sb = load(wts, sq_d, (128, T), bf16, "sq")
            ck_sb = load(wts, ck_d, (128, T), bf16, "ck")
            sk_sb = load(wts, sk_d, (128, T), bf16, "sk")
            ones_c = load(wts, onesc_d, (128, 1), bf16, "onesc")
            ones_r = load(wts, onesr_d, (1, 128), fp32, "onesr")

            # ---- projections: latent^T and q^T (both [128, T]) ----
            latentT = big.tile([128, T], bf16, tag="latentT")
            qT = big.tile([128, T], bf16, tag="qT")
            for dest, w_sb, ptag in ((qT, wq_sb, "sp"), (latentT, wu_sb, "pb")):
                pool = psA if ptag == "sp" else psB
                for tt in range(4):
                    tsl = slice(tt * 512, (tt + 1) * 512)
                    ps = pool.tile([128, 512], fp32, tag=ptag)
                    for kc in range(16):
                        nc.tensor.matmul(
                            ps[:],
                            w_sb[:, kc * 128:(kc + 1) * 128],
                            xc[kc][:, tsl],
                            start=(kc == 0),
                            stop=(kc == 15),
                        )
                    nc.vector.tensor_copy(dest[:, tsl], ps[:])

            # ---- rope helper: out = x*C + shuffle(x)*S ----
            def rope(dst, src, c_sb, s_sb):
                rot = ropep.tile([128, T], bf16, tag="rot")
                nc.vector.stream_shuffle(rot[:], src[:], SWAP_MASK)
                t1 = ropep.tile([128, T], bf16, tag="t1")
                nc.vector.tensor_mul(t1[:], src[:], c_sb[:])
                t2 = ropep.tile([128, T], bf16, tag="t2")
                nc.vector.tensor_mul(t2[:], rot[:], s_sb[:])
                nc.vector.tensor_add(dst[:], t1[:], t2[:])

            Qp = big.tile([128, T], bf16, tag="Qp")
            rope(Qp, qT, cq_sb, sq_sb)

            # ---- K'^T raw = wkvk.T @ latent^T ; then rope ----
            kraw = big.tile([128, T], bf16, tag="kraw")
            for tt in range(4):
                tsl = slice(tt * 512, (tt + 1) * 512)
                ps = psA.tile([128, 512], fp32, tag="sp")
                nc.tensor.matmul(ps[:], wkvk_sb[:], latentT[:, tsl],
                                 start=True, stop=True)
                nc.vector.tensor_copy(kraw[:, tsl], ps[:])
            Kp = big.tile([128, T], bf16, tag="Kp")
            rope(Kp, kraw, ck_sb, sk_sb)

            # ---- V: per k-tile [128k, 512dv] = latent^T-tile.T @ wkvv ----
            v_sb = big.tile([128, 16 * 512], bf16, tag="vall")
            for kt in range(16):
                ps = psB.tile([128, 512], fp32, tag="pb")
                nc.tensor.matmul(ps[:], latentT[:, kt * 128:(kt + 1) * 128],
                                 wkvv_sb[:], start=True, stop=True)
                nc.vector.tensor_copy(v_sb[:, kt * 512:(kt + 1) * 512], ps[:])

            # ---- attention ----
            o_sb = big.tile([128, HL * T], bf16, tag="oall")
            for qb in range(4):
                qsl = slice(qb * 512, (qb + 1) * 512)
                for pair in range(2):
                    heads = (2 * pair, 2 * pair + 1)
                    pv = [psB.tile([128, 512], fp32, tag="pb", name=f"pv{qb}_{pair}_{i}") for i, _ in enumerate(heads)]
                    rp = [psC.tile([1, 512], fp32, tag="pc", name=f"rp{qb}_{pair}_{i}") for i, _ in enumerate(heads)]
                    for kt in range(16):
                        ksl = slice(kt * 128, (kt + 1) * 128)
                        sp = psA.tile([128, 1024], fp32, tag="sp")
                        for idx, j in enumerate(heads):
                            hsl = slice(32 * j, 32 * j + 32)
                            nc.tensor.matmul(
                                sp[:, idx * 512:(idx + 1) * 512],
                                Kp[hsl, ksl], Qp[hsl, qsl],
                                start=True, stop=True,
                                tile_position=(32 * j, 0),
                            )
                        pp = ppool.tile([128, 1024], bf16, tag="pp")
                        nc.scalar.activation(pp[:], sp[:], Exp)
                        for idx, j in enumerate(heads):
                            psl = slice(idx * 512, (idx + 1) * 512)
                            nc.tensor.matmul(
                                pv[idx][:],
                                v_sb[:, kt * 512 + j * 128: kt * 512 + (j + 1) * 128],
                                pp[:, psl],
                                start=(kt == 0), stop=(kt == 15),
                            )
                            nc.tensor.matmul(
                                rp[idx][:], ones_c[:], pp[:, psl],
                                start=(kt == 0), stop=(kt == 15),
                            )
                    # normalize: o = pv / r  (r broadcast over partitions via PE)
                    for idx, j in enumerate(heads):
                        rsb = npool.tile([1, 512], fp32, tag="rsb")
                        nc.vector.tensor_copy(rsb[:], rp[idx][:])
                        Rp = psC.tile([128, 512], fp32, tag="pc")
                        nc.tensor.matmul(Rp[:], ones_r[:], rsb[:],
                                         start=True, stop=True)
                        rrec = npool.tile([128, 512], fp32, tag="rrec")
                        nc.vector.reciprocal_approx_fast(rrec[:], Rp[:])
                        nc.vector.tensor_mul(
                            o_sb[:, j * T + qb * 512: j * T + (qb + 1) * 512],
                            pv[idx][:], rrec[:],
                        )

            # ---- output projection: partial^T[n, t] ----
            for nt in range(16):
                for tt in range(4):
                    pool = psA if (nt * 4 + tt) % 2 == 0 else psB
                    ps = pool.tile([128, 512], fp32,
                                   tag="sp" if pool is psA else "pb")
                    for dc in range(4):
                        nc.tensor.matmul(
                            ps[:],
                            wo_sb[:, dc * T + nt * 128: dc * T + (nt + 1) * 128],
                            o_sb[:, dc * T + tt * 512: dc * T + (tt + 1) * 512],
                            start=(dc == 0), stop=(dc == 3),
                        )
                    ost = npool.tile([128, 512], fp32, tag="ost")
                    nc.scalar.copy(ost[:], ps[:])
                    nc.sync.dma_start(
                        out_d[nt * 128:(nt + 1) * 128, tt * 512:(tt + 1) * 512],
                        ost[:],
                    )

    nc.compile()
    return nc


def _prep_inputs(x, cos, sin, wq, wu, wkv, wo):
    """Host-side sharding/layout prep. Returns per-core in_maps."""
    x = np.asarray(x, dtype=np.float32)
    cos = np.asarray(cos, dtype=np.float32)
    sin = np.asarray(sin, dtype=np.float32)
    wq = np.asarray(wq, dtype=np.float32)
    wu = np.asarray(wu, dtype=np.float32)
    wkv = np.asarray(wkv, dtype=np.float32)
    wo = np.asarray(wo, dtype=np.float32)

    xt = np.ascontiguousarray(x[0].T).astype(BF16)  # [DIM, T]

    # lhsT chunk layout [128, 16*128] for dim-chunked weights
    def chunked(w):  # w: [DIM, 128] -> [128, DIM]
        return np.ascontiguousarray(
            w.reshape(16, 128, 128).transpose(1, 0, 2).reshape(128, DIM)
        ).astype(BF16)

    wu_sb = chunked(wu)

    # rope multiplier tables [128, T]
    C = np.zeros((128, T), np.float32)
    S = np.zeros((128, T), np.float32)
    for j in range(HL):
        C[32 * j: 32 * j + 16, :] = 1.0
        for i in range(D_ROPE // 2):
            C[32 * j + 16 + 2 * i, :] = cos[:, i]
            C[32 * j + 17 + 2 * i, :] = cos[:, i]
            S[32 * j + 16 + 2 * i, :] = -sin[:, i]
            S[32 * j + 17 + 2 * i, :] = sin[:, i]
    cq = (SCALE * C).astype(BF16)
    sq = (SCALE * S).astype(BF16)
    ck = C.astype(BF16)
    sk = S.astype(BF16)

    onesc = np.ones((128, 1), BF16)
    onesr = np.ones((1, 128), np.float32)

    in_maps = []
    for c in range(N_CORES):
        hs = [HL * c + j for j in range(HL)]
        wq_c = chunked(wq[:, 128 * c:128 * (c + 1)])
        wkvk = np.ascontiguousarray(
            np.concatenate([wkv[:, 160 * h:160 * h + D_QK] for h in hs], axis=1)
        ).astype(BF16)
        wkvv = np.ascontiguousarray(
            np.concatenate([wkv[:, 160 * h + D_QK:160 * (h + 1)] for h in hs], axis=1)
        ).astype(BF16)
        wo_c = wo[512 * c:512 * (c + 1), :]  # [512, T]
        wo_sb = np.ascontiguousarray(
            wo_c.reshape(4, 128, T).transpose(1, 0, 2).reshape(128, 4 * T)
        ).astype(BF16)
        in_maps.append({
            "xt": xt, "wq": wq_c, "wu": wu_sb,
            "wkvk": wkvk, "wkvv": wkvv, "wo": wo_sb,
            "cq": cq, "sq": sq, "ck": ck, "sk": sk,
            "onesc": onesc, "onesr": onesr,
        })
    return in_maps


def kernel(x, cos, sin, wq, wu, wkv, wo):
    from concourse import bass_utils

    if "nc" not in _CACHE:
        _CACHE["nc"] = _build_module()
    nc = _CACHE["nc"]

    in_maps = _prep_inputs(x, cos, sin, wq, wu, wkv, wo)
    res = bass_utils.run_bass_kernel_spmd(
        nc, in_maps, core_ids=list(range(N_CORES)),
    )
    _CACHE["last_results"] = res

    total = np.zeros((T, T), np.float64)
    for c in range(N_CORES):
        total += res.results[c]["out"].astype(np.float64)
    out = np.ascontiguousarray(total.T).astype(np.float32)
    return out[None]  # [1, T, T]


# revision 17
# speedup vs baseline: 325.4411x; 325.4411x over previous
"""MLA prefill attention kernel for 8 Trainium2 NeuronCores.

Sharding: tensor-parallel over heads (4 of 32 heads per core), wo sharded on
its input dim; per-core partial outputs are summed on the host (the unshard
step of the row-parallel output projection).

Problem shapes (hardcoded per contract):
  B=1, T=2048, DIM=2048, H=32, D_NOPE=D_ROPE=16, D_QK=32, D_V=128, LATENT=128
"""

import math

import ml_dtypes
import numpy as np

T = 2048
DIM = 2048
H = 32
N_CORES = 8
HL = H // N_CORES  # 4 local heads
D_NOPE, D_ROPE = 16, 16
D_QK = D_NOPE + D_ROPE  # 32
D_V = 128
LAT = 128
ROPE_FACTOR, MSCALE0 = 40.0, 1.0
_ms = 0.1 * MSCALE0 * math.log(ROPE_FACTOR) + 1.0
SCALE = (D_QK**-0.5) * _ms * _ms

BF16 = ml_dtypes.bfloat16
FP8 = ml_dtypes.float8_e4m3

# stream_shuffle source map (within each 32-partition block): rows 0..15
# identity (nope), rows 16..31 swap adjacent pairs (rope x0<->x1).
SWAP_MASK = [i if i < 16 else i ^ 1 for i in range(32)]

OUT_DESCALE = 1.0

_CACHE: dict = {}


def _build_module():
    import concourse.mybir as mybir
    import concourse.tile as tile
    from concourse import bacc

    fp32 = mybir.dt.float32
    bf16 = mybir.dt.bfloat16
    fp8 = mybir.dt.float8e4
    DR = mybir.MatmulPerfMode.DoubleRow
    Exp = mybir.ActivationFunctionType.Exp

    nc = bacc.Bacc("TRN2", target_bir_lowering=False, debug=False,
                   num_devices=N_CORES)

    # DRAM I/O (per-core shards prepared on host)
    xt_d = nc.dram_tensor("xt", (DIM, T), bf16, kind="ExternalInput").ap()
    wq_d = nc.dram_tensor("wq", (128, DIM), bf16, kind="ExternalInput").ap()
    wu_d = nc.dram_tensor("wu", (128, DIM), bf16, kind="ExternalInput").ap()
    wkvk_d = nc.dram_tensor("wkvk", (LAT, HL * D_QK), bf16, kind="ExternalInput").ap()
    wkvv_d = nc.dram_tensor("wkvv", (LAT, HL * D_V), bf16, kind="ExternalInput").ap()
    wo_d = nc.dram_tensor("wo", (128, 4 * T), bf16, kind="ExternalInput").ap()
    cq_d = nc.dram_tensor("cq", (128, T), bf16, kind="ExternalInput").ap()
    sq_d = nc.dram_tensor("sq", (128, T), bf16, kind="ExternalInput").ap()
    ck_d = nc.dram_tensor("ck", (128, T), bf16, kind="ExternalInput").ap()
    sk_d = nc.dram_tensor("sk", (128, T), bf16, kind="ExternalInput").ap()
    onesc_d = nc.dram_tensor("onesc", (128, 1), bf16, kind="ExternalInput").ap()
    onesr_d = nc.dram_tensor("onesr", (1, 128), fp32, kind="ExternalInput").ap()
    out_d = nc.dram_tensor("out", (T, T), fp32, kind="ExternalOutput").ap()

    with tile.TileContext(nc) as tc:
        with (
            tc.tile_pool(name="xin", bufs=1) as xin,
            tc.tile_pool(name="wts", bufs=1) as wts,
            tc.tile_pool(name="big", bufs=1) as big,
            tc.tile_pool(name="rope", bufs=1) as ropep,
            tc.tile_pool(name="pp", bufs=6) as ppool,
            tc.tile_pool(name="norm", bufs=4) as npool,
            tc.tile_pool(name="psA", bufs=2, space="PSUM") as psA,
            tc.tile_pool(name="psB", bufs=2, space="PSUM") as psB,
            tc.tile_pool(name="psC", bufs=1, space="PSUM") as psC,
            tc.tile_pool(name="psD", bufs=1, space="PSUM") as psD,
        ):
            # ---- load inputs (weights first so the projection matmuls
            # can start as soon as the first x^T chunk lands) ----
            def load(pool, dram, shape, dt, tag):
                t = pool.tile(list(shape), dt, tag=tag, name=tag)
                nc.sync.dma_start(t[:], dram[:])
                return t

            wq_sb = load(wts, wq_d, (128, DIM), bf16, "wq")
            wu_sb = load(wts, wu_d, (128, DIM), bf16, "wu")
            wkvk_sb = load(wts, wkvk_d, (LAT, HL * D_QK), bf16, "wkvk")
            wkvv_sb = load(wts, wkvv_d, (LAT, HL * D_V), bf16, "wkvv")
            ones_c = load(wts, onesc_d, (128, 1), bf16, "onesc")
            ones_r = load(wts, onesr_d, (1, 128), fp32, "onesr")
            xc = []
            for kc in range(16):
                t = xin.tile([128, T], bf16, tag=f"xc{kc}")
                nc.sync.dma_start(t[:], xt_d[kc * 128:(kc + 1) * 128, :])
                xc.append(t)
            ck_sb = load(wts, ck_d, (128, T), bf16, "ck")
            sk_sb = load(wts, sk_d, (128, T), bf16, "sk")
            cq_sb = load(wts, cq_d, (128, T), bf16, "cq")
            sq_sb = load(wts, sq_d, (128, T), bf16, "sq")
            wo_sb = load(wts, wo_d, (128, 4 * T), bf16, "wo")

            # ---- projections: latent^T and q^T (both [128, T]) ----
            # kc-outer with 8 concurrent PSUM accumulators (all 8 banks) so
            # PE tracks the x^T chunk DMAs instead of waiting for all 16.
            latentT = big.tile([128, T], bf16, tag="latentT")
            qT = big.tile([128, T], bf16, tag="qT")
            qps = [psA.tile([128, 1024], fp32, tag="sp", name=f"qps{i}")
                   for i in range(2)]
            lps = [psB.tile([128, 512], fp32, tag="pb", name=f"lps{i}")
                   for i in range(2)]
            lps.append(psC.tile([128, 512], fp32, tag="pc", name="lps2"))
            lps.append(psD.tile([128, 512], fp32, tag="pd", name="lps3"))
            for kc in range(16):
                w_k = slice(kc * 128, (kc + 1) * 128)
                for tt in range(4):
                    tsl = slice(tt * 512, (tt + 1) * 512)
                    nc.tensor.matmul(
                        qps[tt // 2][:, (tt % 2) * 512:(tt % 2 + 1) * 512],
                        wq_sb[:, w_k], xc[kc][:, tsl],
                        start=(kc == 0), stop=(kc == 15))
                    nc.tensor.matmul(
                        lps[tt][:], wu_sb[:, w_k], xc[kc][:, tsl],
                        start=(kc == 0), stop=(kc == 15))
            for tt in range(4):
                tsl = slice(tt * 512, (tt + 1) * 512)
                nc.scalar.copy(
                    qT[:, tsl],
                    qps[tt // 2][:, (tt % 2) * 512:(tt % 2 + 1) * 512])
                nc.scalar.copy(latentT[:, tsl], lps[tt][:])

            # ---- rope helper: one 512-col slice ----
            def rope_slice(dst, src, c_sb, s_sb, rtag, tt):
                tsl = slice(tt * 512, (tt + 1) * 512)
                rot = ropep.tile([128, 512], bf16, tag="rot",
                                 name=f"rot{rtag}{tt}")
                nc.vector.stream_shuffle(rot[:], src[:, tsl], SWAP_MASK)
                t1 = ropep.tile([128, 512], bf16, tag="t1",
                                name=f"t1{rtag}{tt}")
                nc.vector.tensor_mul(t1[:], src[:, tsl], c_sb[:, tsl])
                t2 = ropep.tile([128, 512], bf16, tag="t2",
                                name=f"t2{rtag}{tt}")
                nc.vector.tensor_mul(t2[:], rot[:], s_sb[:, tsl])
                nc.vector.tensor_add(dst[:, tsl], t1[:], t2[:])

            # ---- K'^T raw = wkvk.T @ latent^T ; rope K/Q interleaved so
            # the first score matmuls start as early as possible ----
            kraw = big.tile([128, T], bf16, tag="kraw")
            for tt in range(4):
                tsl = slice(tt * 512, (tt + 1) * 512)
                ps = psA.tile([128, 512], fp32, tag="sp")
                nc.tensor.matmul(ps[:], wkvk_sb[:], latentT[:, tsl],
                                 start=True, stop=True)
                nc.scalar.copy(kraw[:, tsl], ps[:])
            Kp = big.tile([128, T], bf16, tag="Kp")
            Qp = big.tile([128, T], bf16, tag="Qp")
            for tt in range(4):
                rope_slice(Kp, kraw, ck_sb, sk_sb, "k", tt)
                rope_slice(Qp, qT, cq_sb, sq_sb, "q", tt)

            # ---- V: per k-tile [128k, 512dv] = latent^T-tile.T @ wkvv ----
            v_sb = big.tile([128, 16 * 512], bf16, tag="vall")
            for kt in range(16):
                ps = psB.tile([128, 512], fp32, tag="pb")
                nc.tensor.matmul(ps[:], latentT[:, kt * 128:(kt + 1) * 128],
                                 wkvv_sb[:], start=True, stop=True)
                nc.scalar.copy(v_sb[:, kt * 512:(kt + 1) * 512], ps[:])

            # ---- attention ----
            o_sb = big.tile([128, HL * T], bf16, tag="oall")
            for qb in range(4):
                qsl = slice(qb * 512, (qb + 1) * 512)
                for pair in range(2):
                    heads = (2 * pair, 2 * pair + 1)
                    pv = [psB.tile([128, 512], fp32, tag="pb", name=f"pv{qb}_{pair}_{i}") for i, _ in enumerate(heads)]
                    r2 = psC.tile([128, 512], fp32, tag="pc",
                                  name=f"r2{qb}_{pair}")
                    rp = [r2[0:1, :], r2[32:33, :]]
                    for kt in range(16):
                        ksl = slice(kt * 128, (kt + 1) * 128)
                        sp = psA.tile([128, 1024], fp32, tag="sp",
                                      name=f"sp{qb}_{pair}_{kt}")
                        for idx, j in enumerate(heads):
                            nc.tensor.matmul(
                                sp[:, idx * 512:(idx + 1) * 512],
                                Kp[32 * j:32 * j + 32, ksl],
                                Qp[32 * j:32 * j + 32, qsl],
                                start=True, stop=True,
                                tile_position=(32 * j, 0),
                            )
                        pp = ppool.tile([128, 1024], bf16, tag="pp",
                                        name=f"pp{qb}_{pair}_{kt}")
                        nc.scalar.activation(pp[:], sp[:], Exp)
                        for idx, j in enumerate(heads):
                            psl = slice(idx * 512, (idx + 1) * 512)
                            nc.tensor.matmul(
                                pv[idx][:],
                                v_sb[:, kt * 512 + j * 128:
                                     kt * 512 + (j + 1) * 128],
                                pp[:, psl],
                                start=(kt == 0), stop=(kt == 15),
                            )
                            nc.tensor.matmul(
                                rp[idx], ones_c[:], pp[:, psl],
                                start=(kt == 0), stop=(kt == 15),
                            )
                    # normalize: o = pv / r  (r broadcast over partitions via PE)
                    for idx, j in enumerate(heads):
                        rsb = npool.tile([1, 512], fp32, tag="rsb",
                                         name=f"rsb{qb}_{pair}_{idx}")
                        nc.vector.tensor_copy(rsb[:], rp[idx])
                        Rp = psD.tile([128, 512], fp32, tag="pd",
                                      name=f"Rp{qb}_{pair}_{idx}")
                        nc.tensor.matmul(Rp[:], ones_r[:], rsb[:],
                                         start=True, stop=True)
                        rrec = npool.tile([128, 512], fp32, tag="rrec")
                        nc.vector.reciprocal_approx_fast(rrec[:], Rp[:])
                        nc.vector.tensor_mul(
                            o_sb[:, j * T + qb * 512: j * T + (qb + 1) * 512],
                            pv[idx][:], rrec[:],
                        )

                # ---- output projection for this t-slice (interleaves with
                # the next q-block's attention on spare psB slots) ----
                for nt in range(16):
                    if qb == 3 and nt % 3 == 1:
                        ps = psB.tile([128, 512], fp32, tag="pb",
                                      name=f"wops{qb}_{nt}")
                    elif qb == 3 and nt % 3 == 2:
                        ps = psA.tile([128, 512], fp32, tag="sp",
                                      name=f"wops{qb}_{nt}")
                    else:
                        ps = psD.tile([128, 512], fp32, tag="pd",
                                      name=f"wops{qb}_{nt}")
                    for dc in range(4):
                        nc.tensor.matmul(
                            ps[:],
                            wo_sb[:, dc * T + nt * 128: dc * T + (nt + 1) * 128],
                            o_sb[:, dc * T + qb * 512: dc * T + (qb + 1) * 512],
                            start=(dc == 0), stop=(dc == 3),
                        )
                    ost = npool.tile([128, 512], fp32, tag="ost",
                                     name=f"ost{qb}_{nt}")
                    if nt % 2 == 0:
                        nc.vector.tensor_copy(ost[:], ps[:])
                    else:
                        nc.scalar.copy(ost[:], ps[:])
                    nc.sync.dma_start(
                        out_d[nt * 128:(nt + 1) * 128,
                              qb * 512:(qb + 1) * 512],
                        ost[:],
                    )

    nc.compile()
    return nc


def _prep_inputs(x, cos, sin, wq, wu, wkv, wo):
    """Host-side sharding/layout prep. Returns per-core in_maps."""
    x = np.asarray(x, dtype=np.float32)
    cos = np.asarray(cos, dtype=np.float32)
    sin = np.asarray(sin, dtype=np.float32)
    wq = np.asarray(wq, dtype=np.float32)
    wu = np.asarray(wu, dtype=np.float32)
    wkv = np.asarray(wkv, dtype=np.float32)
    wo = np.asarray(wo, dtype=np.float32)

    xt = np.ascontiguousarray(x[0].T).astype(BF16)  # [DIM, T]

    # lhsT chunk layout [128, 16*128] for dim-chunked weights
    def chunked(w):  # w: [DIM, 128] -> [128, DIM]
        return np.ascontiguousarray(
            w.reshape(16, 128, 128).transpose(1, 0, 2).reshape(128, DIM)
        ).astype(BF16)

    wu_sb = chunked(wu)

    # rope multiplier tables [128, T]
    C = np.zeros((128, T), np.float32)
    S = np.zeros((128, T), np.float32)
    for j in range(HL):
        C[32 * j: 32 * j + 16, :] = 1.0
        for i in range(D_ROPE // 2):
            C[32 * j + 16 + 2 * i, :] = cos[:, i]
            C[32 * j + 17 + 2 * i, :] = cos[:, i]
            S[32 * j + 16 + 2 * i, :] = -sin[:, i]
            S[32 * j + 17 + 2 * i, :] = sin[:, i]
    cq = (SCALE * C).astype(BF16)
    sq = (SCALE * S).astype(BF16)
    ck = C.astype(BF16)
    sk = S.astype(BF16)

    onesc = np.ones((128, 1), BF16)
    onesr = np.ones((1, 128), np.float32)

    in_maps = []
    for c in range(N_CORES):
        hs = [HL * c + j for j in range(HL)]
        wq_c = chunked(wq[:, 128 * c:128 * (c + 1)])
        wkvk = np.ascontiguousarray(
            np.concatenate([wkv[:, 160 * h:160 * h + D_QK] for h in hs], axis=1)
        ).astype(BF16)
        wkvv = np.ascontiguousarray(
            np.concatenate([wkv[:, 160 * h + D_QK:160 * (h + 1)] for h in hs], axis=1)
        ).astype(BF16)
        wo_c = wo[512 * c:512 * (c + 1), :]  # [512, T]
        wo_sb = np.ascontiguousarray(
            wo_c.reshape(4, 128, T).transpose(1, 0, 2).reshape(128, 4 * T)
        ).astype(BF16)
        in_maps.append({
            "xt": xt, "wq": wq_c, "wu": wu_sb,
            "wkvk": wkvk, "wkvv": wkvv, "wo": wo_sb,
            "cq": cq, "sq": sq, "ck": ck, "sk": sk,
            "onesc": onesc, "onesr": onesr,
        })
    return in_maps


def kernel(x, cos, sin, wq, wu, wkv, wo):
    from concourse import bass_utils

    if "nc" not in _CACHE:
        _CACHE["nc"] = _build_module()
    nc = _CACHE["nc"]

    in_maps = _prep_inputs(x, cos, sin, wq, wu, wkv, wo)
    res = bass_utils.run_bass_kernel_spmd(
        nc, in_maps, core_ids=list(range(N_CORES)),
    )
    _CACHE["last_results"] = res

    total = np.zeros((T, T), np.float64)
    for c in range(N_CORES):
        total += res.results[c]["out"].astype(np.float64)
    out = np.ascontiguousarray(total.T).astype(np.float32)
    return out[None]  # [1, T, T]
